# revision 1
# baseline (speedup 1.0000x reference)
"""CEM sampling kernel for Trainium2, 8-core SPMD (population sharded).

Pipeline per core (512 of 4096 population members):
  1. DTW min-plus DP over [128x128] cost tables via tensor_tensor_scan:
     all 4 population tiles packed into one [128, 516] row buffer with
     +inf separators, so each DP row is 2 DVE ops (shifted min + scan).
  2. AllGather local dists -> global [4096]; rank-count against own
     dists to get the global top-K elite mask without sorting.
  3. Weighted mean / E[x^2] partial sums over own noise shard
     (actions computed in place on ACT+GPSIMD during the DTW window),
     AllReduce partials, finish tiny [128,32] math, write [2,T,1,A].
"""

import os
import sys

for _p in ("/opt/trn_rl_repo", "/root/.axon_site/_ro/trn_rl_repo"):
    if _p not in sys.path:
        sys.path.insert(0, _p)

import numpy as np

import concourse.bass as bass
import concourse.bacc as bacc
import concourse.tile as tile
from concourse import mybir
from concourse import bass_utils

F32 = mybir.dt.float32
ALU = mybir.AluOpType
ACTF = mybir.ActivationFunctionType

P, T, A = 4096, 128, 32
NCORES = 8
PL = P // NCORES          # 512 population per core
NT = PL // 128            # 4 tiles of 128 on the partition dim
S = T + 1                 # 129: segment stride (128 cols + separator)
W = NT * S                # 516: packed row width
K = int(P * 0.1)          # 409
TEMP, MOM, MIN_STD = 0.5, 0.1, 0.05
INF = 1.0e30
RCH = int(os.environ.get("CEM_RCH", "8"))  # DP rows per streamed cost chunk
NCHUNK = T // RCH
GROUP = [list(range(NCORES))]

_CACHE = {}


def _build(stage=9, single=False):
    nc = bacc.Bacc(
        "TRN2",
        target_bir_lowering=False,
        debug=False,
        num_devices=1 if single else NCORES,
    )
    obs_d = nc.dram_tensor("obs", [PL, T, T], F32, kind="ExternalInput")
    means_d = nc.dram_tensor("means", [T, 1, A], F32, kind="ExternalInput")
    stds_d = nc.dram_tensor("stds", [T, 1, A], F32, kind="ExternalInput")
    noise_d = nc.dram_tensor("noise", [T, PL, A], F32, kind="ExternalInput")
    out_d = nc.dram_tensor("out", [2, T, 1, A], F32, kind="ExternalOutput")

    with tile.TileContext(nc) as tc:
        with (
            tc.tile_pool(name="main", bufs=1) as mp,
            tc.tile_pool(name="cwin", bufs=int(os.environ.get("CEM_CBUFS", "3"))) as cp,
            tc.tile_pool(name="dram", bufs=1, space="DRAM") as dp,
        ):
            # ---- stats-stage tiles; DMA early so actions overlap DTW
            noise_t = mp.tile([T, PL, A], F32)
            means_t = mp.tile([T, A], F32)
            stds_t = mp.tile([T, A], F32)
            nc.sync.dma_start(means_t[:], means_d[:, 0, :])
            nc.sync.dma_start(stds_t[:], stds_d[:, 0, :])

            def _actions_block():
                # noise prefetch + actions = clip(means + stds * noise) in
                # place, per action dim. ACT does the affine, GPSIMD the
                # clip: both idle during DTW. Traced after the first cost
                # chunks so their DMAs win the queue race.
                nc.sync.dma_start(noise_t[:], noise_d[:, :, :])
                if stage < 1:
                    return
                for a in range(A):
                    sl = noise_t[:, :, a]
                    nc.scalar.activation(
                        sl,
                        sl,
                        ACTF.Identity,
                        bias=means_t[:, a : a + 1],
                        scale=stds_t[:, a : a + 1],
                    )
                    nc.gpsimd.tensor_scalar(
                        sl, sl, 1.0, -1.0, op0=ALU.min, op1=ALU.max
                    )

            # ---- DTW DP over packed rows
            pbuf = mp.tile([128, W + 1], F32)
            ubuf = mp.tile([128, W], F32)
            nc.vector.memset(pbuf[:], INF)
            for k in range(NT):
                nc.vector.memset(pbuf[:, k * S : k * S + 1], 0.0)

            chunk_rows = [RCH] * (T // RCH)
            assert sum(chunk_rows) == T
            r0 = 0
            for c, rows in enumerate(chunk_rows):
                cb = cp.tile([128, rows, NT, S], F32, tag="cw")
                for k in range(NT):
                    nc.sync.dma_start(
                        cb[:, :, k, 0:T],
                        obs_d[k * 128 : (k + 1) * 128, r0 : r0 + rows, :],
                    )
                nc.gpsimd.memset(cb[:, :, :, T:S], INF)
                if c == 2:
                    _actions_block()
                for r in range(rows):
                    crow = cb[:, r].rearrange("p k j -> p (k j)")
                    nc.vector.tensor_tensor(
                        ubuf[:], pbuf[:, 0:W], pbuf[:, 1 : W + 1], op=ALU.min
                    )
                    nc.vector.tensor_tensor_scan(
                        pbuf[:, 1 : W + 1],
                        ubuf[:],
                        crow,
                        INF,
                        op0=ALU.min,
                        op1=ALU.add,
                    )
                    if c == 0 and r == 0:
                        # tile 0's left-boundary slot is never rewritten by
                        # the scan; after row 0 it must be +inf (D[i,0]).
                        nc.vector.memset(pbuf[:, 0:1], INF)
                r0 += rows

            # own dists: last col of each packed segment -> [128, NT]
            down = mp.tile([128, NT], F32)
            for k in range(NT):
                nc.vector.tensor_copy(
                    down[:, k : k + 1], pbuf[:, k * S + T : k * S + T + 1]
                )

            if stage >= 2:
                # ---- AllGather dists (tiny)
                ld = dp.tile([PL], F32)
                gd = dp.tile([P], F32)
                nc.sync.dma_start(ld.rearrange("(k p) -> p k", p=128), down[:])
                if single:
                    for cc in range(NCORES):
                        nc.sync.dma_start(gd[cc * PL : (cc + 1) * PL], ld[:])
                else:
                    nc.gpsimd.collective_compute(
                        "AllGather",
                        ALU.bypass,
                        replica_groups=GROUP,
                        ins=[ld.opt()],
                        outs=[gd.opt()],
                    )

            if stage >= 3:
                # broadcast global dists across partitions: 0-stride DMA
                # re-reads the 16KB vector once per partition; two halves so
                # the rank compares overlap the second half's transfer
                PH = P // 2
                gdb = mp.tile([128, 2, PH], F32)
                for h in range(2):
                    _, gsrc = bass.broadcast_tensor_aps(
                        gdb[:, h],
                        gd[h * PH : (h + 1) * PH].rearrange("(o f) -> o f", o=1),
                    )
                    nc.sync.dma_start(gdb[:, h], gsrc)

                ming2 = mp.tile([128, 2], F32)
                ming = mp.tile([128, 1], F32)
                # rank of own dists = #(d_j < d_p) per half; elite iff sum < K
                rank8 = mp.tile([128, 2, NT], F32)
                rank4 = mp.tile([128, NT], F32)
                scratch = cp.tile([128, PH], F32, tag="cw")
                for h in range(2):
                    for k in range(NT):
                        nc.vector.tensor_scalar(
                            scratch[:],
                            gdb[:, h],
                            down[:, k : k + 1],
                            None,
                            op0=ALU.is_lt,
                            op1=ALU.add,
                            accum_out=rank8[:, h, k : k + 1],
                        )
                    nc.vector.tensor_reduce(
                        ming2[:, h : h + 1],
                        gdb[:, h],
                        axis=mybir.AxisListType.X,
                        op=ALU.min,
                    )
                nc.vector.tensor_tensor(
                    rank4[:], rank8[:, 0], rank8[:, 1], op=ALU.add
                )
                nc.vector.tensor_reduce(
                    ming[:], ming2[:], axis=mybir.AxisListType.X, op=ALU.min
                )
                mask4 = mp.tile([128, NT], F32)
                nc.vector.tensor_scalar(
                    mask4[:], rank4[:], float(K), None, op0=ALU.is_lt
                )

                # w = mask * exp(TEMP*(min - d))
                biast = mp.tile([128, 1], F32)
                nc.vector.tensor_scalar(biast[:], ming[:], TEMP, None, op0=ALU.mult)
                e4 = mp.tile([128, NT], F32)
                nc.scalar.activation(
                    e4[:], down[:], ACTF.Exp, bias=biast[:, 0:1], scale=-TEMP
                )
                w4 = mp.tile([128, NT], F32)
                nc.vector.tensor_tensor(w4[:], e4[:], mask4[:], op=ALU.mult)

                # broadcast own weights along partitions: [128, PL]
                wl = dp.tile([PL], F32)
                nc.sync.dma_start(wl.rearrange("(k p) -> p k", p=128), w4[:])
                wrow = mp.tile([128, PL], F32)
                _, wsrc = bass.broadcast_tensor_aps(
                    wrow[:], wl.rearrange("(o f) -> o f", o=1)
                )
                nc.sync.dma_start(wrow[:], wsrc)

                slocal = mp.tile([128, 1], F32)
                nc.vector.tensor_reduce(
                    slocal[:], wrow[:], axis=mybir.AxisListType.X, op=ALU.add
                )

            if stage >= 4:
                # ---- weighted partial sums over own shard, in a-halves:
                # wa = w*act (broadcast w along a), act^2*w in place over
                # noise, then one strided reduce over p per quantity.
                num1 = mp.tile([128, A], F32)
                num2 = mp.tile([128, A], F32)
                AH = A // 2
                wah = cp.tile([128, PL, AH], F32, tag="wah", bufs=1)
                waa_dump = mp.tile([128, PL], F32)
                wrow3 = wrow[:].rearrange("t (p o) -> t p o", o=1)
                DSPL = 8  # a-columns per half on DVE; rest on GPSIMD
                for h in range(2):
                    a0 = h * AH
                    for eng, lo, hi in (
                        (nc.vector, 0, DSPL),
                        (nc.gpsimd, DSPL, AH),
                    ):
                        na = noise_t[:, :, a0 + lo : a0 + hi]
                        wv = wah[:, :, lo:hi]
                        b0, b1 = bass.broadcast_tensor_aps(na, wrow3)
                        eng.tensor_tensor(wv, b0, b1, op=ALU.mult)
                        eng.tensor_tensor(na, wv, na, op=ALU.mult)
                    nc.vector.tensor_reduce(
                        num1[:, a0 : a0 + AH],
                        wah[:].rearrange("t p a -> t a p"),
                        axis=mybir.AxisListType.X,
                        op=ALU.add,
                    )
                    for a in range(a0, a0 + AH):
                        nc.scalar.activation(
                            waa_dump[:],
                            noise_t[:, :, a],
                            ACTF.Identity,
                            accum_out=num2[:, a : a + 1],
                        )

            if stage >= 5:
                # ---- AllReduce partials: [num1 | num2 | S]
                NTOT = 2 * T * A + T
                arin = dp.tile([NTOT], F32)
                arout = dp.tile([NTOT], F32)
                nc.sync.dma_start(
                    arin[0 : T * A].rearrange("(p a) -> p a", a=A), num1[:]
                )
                nc.sync.dma_start(
                    arin[T * A : 2 * T * A].rearrange("(p a) -> p a", a=A), num2[:]
                )
                nc.sync.dma_start(
                    arin[2 * T * A : NTOT].rearrange("(p a) -> p a", a=1), slocal[:]
                )
                if single:
                    nc.sync.dma_start(arout[:], arin[:])
                else:
                    nc.gpsimd.collective_compute(
                        "AllReduce",
                        ALU.add,
                        replica_groups=GROUP,
                        ins=[arin.opt()],
                        outs=[arout.opt()],
                    )
                rn12 = mp.tile([128, 2, A], F32)
                rs = mp.tile([128, 1], F32)
                nc.sync.dma_start(
                    rn12[:],
                    arout[0 : 2 * T * A].rearrange("(q p a) -> p q a", q=2, a=A),
                )
                rn1 = rn12[:, 0]
                rn2 = rn12[:, 1]
                nc.sync.dma_start(
                    rs[:], arout[2 * T * A : NTOT].rearrange("(p a) -> p a", a=1)
                )

                # ---- final statistics
                inv = mp.tile([128, 1], F32)
                nc.vector.reciprocal(inv[:], rs[:])
                mh = mp.tile([128, A], F32)
                nc.vector.tensor_scalar(
                    mh[:], rn1, inv[:, 0:1], None, op0=ALU.mult
                )
                q = mp.tile([128, A], F32)
                nc.vector.tensor_scalar(
                    q[:], rn2, inv[:, 0:1], None, op0=ALU.mult
                )
                msq = mp.tile([128, A], F32)
                nc.vector.tensor_tensor(msq[:], mh[:], mh[:], op=ALU.mult)
                var = mp.tile([128, A], F32)
                nc.vector.tensor_tensor(var[:], q[:], msq[:], op=ALU.subtract)
                nc.vector.tensor_scalar(var[:], var[:], 0.0, None, op0=ALU.max)
                stdv = mp.tile([128, A], F32)
                nc.scalar.sqrt(stdv[:], var[:])
                nc.vector.tensor_scalar(
                    stdv[:], stdv[:], MIN_STD, 1.0, op0=ALU.max, op1=ALU.min
                )
                mnew = mp.tile([128, A], F32)
                nc.vector.tensor_scalar(
                    mh[:], mh[:], 1.0 - MOM, None, op0=ALU.mult
                )
                nc.vector.scalar_tensor_tensor(
                    mnew[:], means_t[:], MOM, mh[:], op0=ALU.mult, op1=ALU.add
                )
                nc.sync.dma_start(out_d[0, :, 0, :], mnew[:])
                nc.sync.dma_start(out_d[1, :, 0, :], stdv[:])
            else:
                # bisect debug output
                dbg = mp.tile([128, A], F32)
                nc.vector.memset(dbg[:], 0.0)
                if stage >= 3:
                    nc.vector.tensor_copy(dbg[:, 0:NT], w4[:])
                    nc.vector.tensor_copy(dbg[:, NT : NT + 1], slocal[:])
                elif stage >= 0:
                    nc.vector.tensor_copy(dbg[:, 0:NT], down[:])
                if stage == 2:
                    gdbg = mp.tile([128, A], F32)
                    nc.sync.dma_start(
                        gdbg[:],
                        gd[0 : 128 * A].rearrange("(p a) -> p a", a=A),
                    )
                    nc.vector.tensor_copy(dbg[:, 4:8], gdbg[:, 0:4])
                nc.sync.dma_start(out_d[0, :, 0, :], dbg[:])
                nc.sync.dma_start(out_d[1, :, 0, :], dbg[:])

    nc.compile()
    return nc


def _get_nc(stage=None, single=None):
    # staged/single variants exist only for the dev harness (test.py);
    # kernel() always runs the full 8-core program.
    if stage is None:
        stage = int(os.environ.get("CEM_STAGE", "9"))
    if single is None:
        single = bool(int(os.environ.get("CEM_SINGLE", "0")))
    key = ("nc", stage, single)
    if key not in _CACHE:
        _CACHE[key] = _build(stage, single)
    return _CACHE[key]


def kernel(**inputs):
    obs = np.ascontiguousarray(np.asarray(inputs["obs_diffs"], np.float32))
    means = np.ascontiguousarray(np.asarray(inputs["means"], np.float32))
    stds = np.ascontiguousarray(np.asarray(inputs["stds"], np.float32))
    noise = np.ascontiguousarray(np.asarray(inputs["noise"], np.float32))

    nc = _get_nc(stage=9, single=False)
    in_maps = []
    for c in range(NCORES):
        in_maps.append(
            {
                "obs": obs[c * PL : (c + 1) * PL],
                "means": means,
                "stds": stds,
                "noise": np.ascontiguousarray(noise[:, c * PL : (c + 1) * PL, :]),
            }
        )
    res = bass_utils.run_bass_kernel_spmd(
        nc, in_maps, core_ids=list(range(NCORES))
    )
    out = np.asarray(res.results[0]["out"], np.float32)
    return out.reshape(2, T, 1, A)



# revision 46
# speedup vs baseline: 1.1998x; 1.1998x over previous
"""CEM sampling kernel for Trainium2, 8-core SPMD (population sharded).

Per core (512 of 4096 members) the DTW min-plus DP runs on DVE as two
independent packed 257-wide chains (two 128-member lanes each, +inf
separator column), ops interleaved per row so each chain's write-ack
latency hides under the other chain's ops (min-TT and the scan are
DVE-only opcodes on real TRN2; GPSIMD only has add/mult software TTs).
Chunked cost DMAs are emitted per chunk-group so arena-rotation
WAR/RAW deps bind to the right transfers.

The clip in actions = clip(means + stds*noise) is removed
algebraically: ACT computes z = relu(2 - relu(stds*n + means + 1))
(actions = 1 - z) into an f16 transposed [T, A, PL] layout during the
DP window, and the statistics are rebuilt from U = sum(w),
V = sum(w z), Q = sum(w z^2) after the AllReduce: num1 = U - V,
num2 = U - 2V + Q.  Top-K: f16 AllGather of dists, 0-stride broadcast,
rank = #(d_j < d_own) via f16 is_lt compares against the own f32 dist.
"""

import os
import sys

for _p in ("/opt/trn_rl_repo", "/root/.axon_site/_ro/trn_rl_repo"):
    if _p not in sys.path:
        sys.path.insert(0, _p)

import numpy as np

import concourse.bass as bass
import concourse.bacc as bacc
import concourse.tile as tile
from concourse import mybir
from concourse import bass_utils

F32 = mybir.dt.float32
F16 = mybir.dt.float16
ALU = mybir.AluOpType
ACTF = mybir.ActivationFunctionType

P, T, A = 4096, 128, 32
NCORES = 8
PL = P // NCORES          # 512 population per core
K = int(P * 0.1)          # 409
TEMP, MOM, MIN_STD = 0.5, 0.1, 0.05
INF = 1.0e30

RCH = 8
CHUNK_ROWS = [2, 2, 4] + [RCH] * ((T - 8) // RCH)
NB01 = int(os.environ.get("CEM_NB01", "3"))
NB23 = int(os.environ.get("CEM_NB23", "3"))
W2 = 2 * T + 1            # 257 packed pair width (128 | sep | 128)
SWA = int(os.environ.get("CEM_SWA", "80"))   # pair A: DVE rows < SWA, Pool rest
SWB = int(os.environ.get("CEM_SWB", "48"))   # pair B: Pool rows < SWB, DVE rest
NOISE_C0 = int(os.environ.get("CEM_NOISE_C0", "5"))  # first chunk group carrying a noise slice
NY = int(os.environ.get("CEM_NY", "4"))      # stats cols DVE-fed + ACT-accum
NZ = int(os.environ.get("CEM_NZ", "11"))     # stats cols Pool-fed + ACT-accum
GROUP = [list(range(NCORES))]

_CACHE = {}


def _build(stage=9, single=False):
    nc = bacc.Bacc(
        "TRN2",
        target_bir_lowering=False,
        debug=False,
        num_devices=1 if single else NCORES,
    )
    obs_d = nc.dram_tensor("obs", [PL, T, T], F32, kind="ExternalInput")
    means_d = nc.dram_tensor("means", [T, 1, A], F32, kind="ExternalInput")
    stds_d = nc.dram_tensor("stds", [T, 1, A], F32, kind="ExternalInput")
    noise_d = nc.dram_tensor("noise", [T, PL, A], F32, kind="ExternalInput")
    out_d = nc.dram_tensor("out", [2, T, 1, A], F32, kind="ExternalOutput")

    with tile.TileContext(nc) as tc:
        with (
            tc.tile_pool(name="main", bufs=1) as mp,
            tc.tile_pool(name="dram", bufs=1, space="DRAM") as dp,
        ):
            # ---------------- tiles
            means_t = mp.tile([T, A], F32)
            stds_t = mp.tile([T, A], F32)
            bias1_t = mp.tile([T, A], F32)       # means + 1
            two_t = mp.tile([T, 1], F32)
            noise_t = mp.tile([T, PL, A], F32)
            z_t = mp.tile([T, A, PL], F16)       # z = relu(2 - relu(s*n+m+1))
            y_ring = mp.tile([T, 2, PL], F16)

            ar01 = mp.tile([128, NB01, RCH, W2], F32)  # lanes 0,1 packed chunks
            ar23 = mp.tile([128, NB23, RCH, W2], F32)  # lanes 2,3 packed chunks
            pbuf_d = mp.tile([128, W2 + 1], F32)       # DVE packed state
            pbuf_p = mp.tile([128, W2 + 1], F32)       # Pool packed state
            ubuf_d = mp.tile([128, W2], F32)
            ubuf_p = mp.tile([128, W2], F32)

            down32 = mp.tile([128, 4], F32)
            down16 = mp.tile([128, 4], F16)
            gdb = mp.tile([128, 2, P // 2], F16)
            rank8 = mp.tile([128, 2, 4], F32)
            rank4 = mp.tile([128, 4], F32)
            mask4 = mp.tile([128, 4], F32)
            gsb = mp.tile([128, P // 128], F16)
            pmin = mp.tile([128, 1], F16)
            pmb = mp.tile([128, 128], F16)
            ming = mp.tile([128, 1], F16)
            biast = mp.tile([128, 1], F32)
            e4 = mp.tile([128, 4], F32)
            w4 = mp.tile([128, 4], F16)
            epre = mp.tile([128, 1], F32)
            wrow = mp.tile([128, PL], F16)

            wz_d = mp.tile([128, PL], F16)
            wzz_d = mp.tile([128, PL], F16)
            wz_p = mp.tile([128, 4, PL], F16)
            wzz_p = mp.tile([128, 4, PL], F16)
            wz_r = mp.tile([128, 4, PL], F16)
            wzz_r = mp.tile([128, 4, PL], F16)
            dump = mp.tile([128, PL], F16)
            vqu = mp.tile([128, 128], F32)  # blk1: vq-pairs 0..29, U, pad
            rvqu = mp.tile([128, 128], F32)
            vqu2 = mp.tile([128, 128], F32)  # blk2: vq-pairs 30,31, pad
            rvqu2 = mp.tile([128, 128], F32)
            scr16 = wz_r.rearrange("t r p -> t (r p)")  # rank scratch view

            ld16 = dp.tile([PL], F16)
            wl16 = dp.tile([PL], F16)
            gd16 = dp.tile([P], F16)
            pd16 = dp.tile([128], F16)
            arin = dp.tile([2 * 128 * 128], F32)
            arout = dp.tile([2 * 128 * 128], F32)

            # ---------------- init (before chunk DMAs: arena dep direction)
            nc.vector.memset(ar01[:, :, :, T : T + 1], INF)
            nc.vector.memset(pbuf_d[:], INF)
            nc.vector.memset(pbuf_d[:, 0:1], 0.0)
            nc.vector.memset(pbuf_d[:, T + 1 : T + 2], 0.0)
            nc.gpsimd.memset(ar23[:, :, :, T : T + 1], INF)
            nc.vector.memset(pbuf_p[:], INF)
            nc.vector.memset(pbuf_p[:, 0:1], 0.0)
            nc.vector.memset(pbuf_p[:, T + 1 : T + 2], 0.0)
            nc.vector.memset(two_t[:], 2.0)

            # scalar-queue DMAs: means/stds then noise slices
            nc.scalar.dma_start(means_t[:], means_d[:, 0, :])
            nc.scalar.dma_start(stds_t[:], stds_d[:, 0, :])
            nc.vector.tensor_scalar(bias1_t[:], means_t[:], 1.0, None, op0=ALU.add)
            NSL = 8
            PSL = PL // NSL

            # ---------------- DP: per-chunk groups [DMA; pairA rows; pairB]
            # Both engines always run packed-pair ops; ownership swaps once
            # (DVE: A rows <SWA then B rows >=SWB; Pool: B rows <SWB then A).
            def pair_row(eng, pbuf, ubuf, ar, b, rr, r):
                eng.tensor_tensor(
                    ubuf[:], pbuf[:, 0:W2], pbuf[:, 1 : W2 + 1], op=ALU.min
                )
                eng.tensor_tensor_scan(
                    pbuf[:, 1 : W2 + 1], ubuf[:], ar[:, b, rr, :], INF,
                    op0=ALU.min, op1=ALU.add,
                )
                if r == 0:
                    eng.memset(pbuf[:, 0:1], INF)

            # DP runs entirely on DVE (TT-min and the scan are DVE-only
            # opcodes on real TRN2): two packed 257-wide pairs, ops
            # interleaved per row so each chain's write-ack latency hides
            # under the other chain's ops.
            row0 = [0]
            for c, rows in enumerate(CHUNK_ROWS):
                row0.append(row0[-1] + rows)
            NCH = len(CHUNK_ROWS)

            for g in range(NCH):
                rows, r0 = CHUNK_ROWS[g], row0[g]
                b01 = g % NB01
                b23 = g % NB23
                nc.sync.dma_start(
                    ar01[:, b01, 0:rows, 0:T],
                    obs_d[0:128, r0 : r0 + rows, :],
                )
                nc.sync.dma_start(
                    ar01[:, b01, 0:rows, T + 1 : W2],
                    obs_d[128:256, r0 : r0 + rows, :],
                )
                nc.sync.dma_start(
                    ar23[:, b23, 0:rows, 0:T],
                    obs_d[256:384, r0 : r0 + rows, :],
                )
                nc.sync.dma_start(
                    ar23[:, b23, 0:rows, T + 1 : W2],
                    obs_d[384:512, r0 : r0 + rows, :],
                )
                if NOISE_C0 <= g < NOISE_C0 + NSL:
                    s = g - NOISE_C0
                    nc.sync.dma_start(
                        noise_t[:, s * PSL : (s + 1) * PSL, :],
                        noise_d[:, s * PSL : (s + 1) * PSL, :],
                    )
                for rr in range(rows):
                    r = r0 + rr
                    nc.vector.tensor_tensor(
                        ubuf_d[:], pbuf_d[:, 0:W2], pbuf_d[:, 1 : W2 + 1],
                        op=ALU.min,
                    )
                    nc.vector.tensor_tensor(
                        ubuf_p[:], pbuf_p[:, 0:W2], pbuf_p[:, 1 : W2 + 1],
                        op=ALU.min,
                    )
                    nc.vector.tensor_tensor_scan(
                        pbuf_d[:, 1 : W2 + 1], ubuf_d[:], ar01[:, b01, rr, :],
                        INF, op0=ALU.min, op1=ALU.add,
                    )
                    nc.vector.tensor_tensor_scan(
                        pbuf_p[:, 1 : W2 + 1], ubuf_p[:], ar23[:, b23, rr, :],
                        INF, op0=ALU.min, op1=ALU.add,
                    )
                    if r == 0:
                        nc.vector.memset(pbuf_d[:, 0:1], INF)
                        nc.vector.memset(pbuf_p[:, 0:1], INF)

            # ---------------- ACT program: z = relu(2 - relu(s*n + m + 1))
            # two p-halves so the first half starts once noise slices 0-3
            # have landed (the second half's slices arrive mid-window)
            if stage >= 1:
                PH = PL // 2
                for h in range(2):
                    for a in range(A):
                        nc.scalar.activation(
                            y_ring[:, a % 2, 0:PH],
                            noise_t[:, h * PH : (h + 1) * PH, a],
                            ACTF.Relu,
                            bias=bias1_t[:, a : a + 1],
                            scale=stds_t[:, a : a + 1],
                        )
                        nc.scalar.activation(
                            z_t[:, a, h * PH : (h + 1) * PH],
                            y_ring[:, a % 2, 0:PH],
                            ACTF.Relu,
                            bias=two_t[:, 0:1],
                            scale=-1.0,
                        )
                # preload Exp+Sqrt tables while the allgather is in flight
                nc.scalar.activation(epre[:], two_t[:], ACTF.Exp, bias=0.0, scale=1.0)
                nc.scalar.sqrt(epre[:], two_t[:])

            # ---------------- dists -> allgather -> broadcast -> ranks
            nc.vector.tensor_copy(down32[:, 0:1], pbuf_d[:, T : T + 1])
            nc.vector.tensor_copy(down32[:, 1:2], pbuf_d[:, W2 : W2 + 1])
            nc.vector.tensor_copy(down32[:, 2:3], pbuf_p[:, T : T + 1])
            nc.vector.tensor_copy(down32[:, 3:4], pbuf_p[:, W2 : W2 + 1])
            nc.vector.tensor_copy(down16[:], down32[:])

            nc.sync.dma_start(ld16.rearrange("(k p) -> p k", p=128), down16[:])
            if single:
                _, gsrc8 = bass.broadcast_tensor_aps(
                    gd16.rearrange("(r f) -> r f", r=NCORES),
                    ld16.rearrange("(o f) -> o f", o=1),
                )
                nc.sync.dma_start(
                    gd16.rearrange("(r f) -> r f", r=NCORES), gsrc8
                )
            else:
                nc.gpsimd.collective_compute(
                    "AllGather",
                    ALU.bypass,
                    replica_groups=GROUP,
                    ins=[ld16.opt()],
                    outs=[gd16.opt()],
                )

            if stage >= 3:
                PH2 = P // 2
                # global-min chain first in the queue (tiny, unblocks pmin so
                # the pd/pmb round-trip hides under the rank compares)
                nc.sync.dma_start(
                    gsb[:], gd16.rearrange("(p q) -> p q", p=128)
                )
                for h in range(2):
                    _, gsrc = bass.broadcast_tensor_aps(
                        gdb[:, h],
                        gd16[h * PH2 : (h + 1) * PH2].rearrange(
                            "(o f) -> o f", o=1
                        ),
                    )
                    nc.sync.dma_start(gdb[:, h], gsrc)
                nc.vector.tensor_reduce(
                    pmin[:], gsb[:], axis=mybir.AxisListType.X, op=ALU.min
                )
                nc.sync.dma_start(
                    pd16.rearrange("(p q) -> p q", q=1), pmin[:]
                )
                _, psrc = bass.broadcast_tensor_aps(
                    pmb[:], pd16.rearrange("(o f) -> o f", o=1)
                )
                nc.sync.dma_start(pmb[:], psrc)

                for h in range(2):
                    for k in range(4):
                        nc.vector.tensor_scalar(
                            scr16[:, 0 : PH2],
                            gdb[:, h],
                            down32[:, k : k + 1],
                            None,
                            op0=ALU.is_lt,
                            op1=ALU.add,
                            accum_out=rank8[:, h, k : k + 1],
                        )
                nc.vector.tensor_reduce(
                    ming[:], pmb[:], axis=mybir.AxisListType.X, op=ALU.min
                )
                nc.vector.tensor_scalar(
                    biast[:], ming[:], TEMP, None, op0=ALU.mult
                )
                nc.vector.tensor_tensor(
                    rank4[:], rank8[:, 0], rank8[:, 1], op=ALU.add
                )
                nc.vector.tensor_scalar(
                    mask4[:], rank4[:], float(K), None, op0=ALU.is_lt
                )
                nc.scalar.activation(
                    e4[:], down32[:], ACTF.Exp, bias=biast[:, 0:1], scale=-TEMP
                )
                nc.vector.tensor_tensor(w4[:], e4[:], mask4[:], op=ALU.mult)

                nc.scalar.dma_start(wl16.rearrange("(k p) -> p k", p=128), w4[:])
                _, wsrc = bass.broadcast_tensor_aps(
                    wrow[:], wl16.rearrange("(o f) -> o f", o=1)
                )
                nc.scalar.dma_start(wrow[:], wsrc)

            if stage >= 5:
                # vqu slot map: col a -> (base, off): interleaved V/Q pairs
                def V(a):
                    if a < 30:
                        return vqu[:, 2 * a : 2 * a + 1]
                    return vqu2[:, 2 * (a - 30) : 2 * (a - 30) + 1]

                def Q(a):
                    if a < 30:
                        return vqu[:, 2 * a + 1 : 2 * a + 2]
                    return vqu2[:, 2 * (a - 30) + 1 : 2 * (a - 30) + 2]

                U = vqu[:, 60:61]
                nc.vector.tensor_scalar(
                    dump[:], wrow[:], 0.0, None, op0=ALU.add, op1=ALU.add,
                    accum_out=U,
                )
                # fed cols (ACT accumulates): DVE-fed {0..NY-1}, Pool-fed
                # {NY..12, 30, 31}; DVE-solo cols 13..29 finish early so
                # block-1 (cols 0..29 + U) can AllReduce while ACT finishes
                # cols 30,31.
                fedD = list(range(0, NY))
                fedP = list(range(NY, 13))
                fedP2 = [30, 31]
                solo = list(range(13, 30))
                def pool_fed(cols, base):
                    for i0_ in range(0, len(cols), 2):
                        pair = cols[i0_ : i0_ + 2]
                        for j, a in enumerate(pair):
                            nc.gpsimd.tensor_tensor(
                                wz_p[:, (base + i0_ + j) % 4], z_t[:, a, :],
                                wrow[:], op=ALU.mult,
                            )
                        for j, a in enumerate(pair):
                            nc.gpsimd.tensor_tensor(
                                wzz_p[:, (base + i0_ + j) % 4],
                                wz_p[:, (base + i0_ + j) % 4],
                                z_t[:, a, :], op=ALU.mult,
                            )
                        for j, a in enumerate(pair):
                            nc.scalar.activation(
                                dump[:], wz_p[:, (base + i0_ + j) % 4],
                                ACTF.Identity, accum_out=V(a),
                            )
                            nc.scalar.activation(
                                dump[:], wzz_p[:, (base + i0_ + j) % 4],
                                ACTF.Identity, accum_out=Q(a),
                            )

                for a in fedD:
                    za = z_t[:, a, :]
                    nc.vector.tensor_tensor(
                        wz_r[:, a % 4], za, wrow[:], op=ALU.mult
                    )
                    nc.vector.tensor_tensor(
                        wzz_r[:, a % 4], wz_r[:, a % 4], za, op=ALU.mult
                    )
                    nc.scalar.activation(
                        dump[:], wz_r[:, a % 4], ACTF.Identity, accum_out=V(a)
                    )
                    nc.scalar.activation(
                        dump[:], wzz_r[:, a % 4], ACTF.Identity, accum_out=Q(a)
                    )
                pool_fed(fedP, 0)
                pool_fed(fedP2, len(fedP))
                dump2 = wzz_d
                for a in solo:
                    za = z_t[:, a, :]
                    nc.vector.tensor_tensor(wz_d[:], za, wrow[:], op=ALU.mult)
                    nc.vector.tensor_scalar(
                        dump2[:], wz_d[:], 0.0, None, op0=ALU.add, op1=ALU.add,
                        accum_out=V(a),
                    )
                    nc.vector.tensor_tensor(wz_d[:], wz_d[:], za, op=ALU.mult)
                    nc.vector.tensor_scalar(
                        dump2[:], wz_d[:], 0.0, None, op0=ALU.add, op1=ALU.add,
                        accum_out=Q(a),
                    )

                # ---------------- split AllReduce: block-1 (cols 0..29 +
                # U) fires while ACT still accumulates cols 30,31; the tiny
                # block-2 collective follows with a short final chain.
                nc.vector.memset(vqu[:, 61:128], 0.0)
                nc.vector.memset(vqu2[:, 4:128], 0.0)
                nc.sync.dma_start(
                    arin[0 : 128 * 128].rearrange("(p a) -> p a", a=128),
                    vqu[:],
                )
                if single:
                    nc.sync.dma_start(arout[0 : 128 * 128], arin[0 : 128 * 128])
                else:
                    nc.gpsimd.collective_compute(
                        "AllReduce", ALU.add, replica_groups=GROUP,
                        ins=[arin[0 : 128 * 128].opt()],
                        outs=[arout[0 : 128 * 128].opt()],
                    )
                nc.sync.dma_start(
                    rvqu[:],
                    arout[0 : 128 * 128].rearrange("(p a) -> p a", a=128),
                )
                nc.sync.dma_start(
                    arin[128 * 128 :].rearrange("(p a) -> p a", a=128),
                    vqu2[:],
                )
                if single:
                    nc.sync.dma_start(arout[128 * 128 :], arin[128 * 128 :])
                else:
                    nc.gpsimd.collective_compute(
                        "AllReduce", ALU.add, replica_groups=GROUP,
                        ins=[arin[128 * 128 :].opt()],
                        outs=[arout[128 * 128 :].opt()],
                    )
                nc.sync.dma_start(
                    rvqu2[:],
                    arout[128 * 128 :].rearrange("(p a) -> p a", a=128),
                )

                # ---------------- final statistics, block-2 math first
                inv = mp.tile([128, 1], F32)
                mh = mp.tile([128, A], F32)
                mstd = mp.tile([128, 2, A], F32)
                q = mp.tile([128, A], F32)
                t2 = mp.tile([128, A], F32)
                msq = mp.tile([128, A], F32)
                var = mp.tile([128, A], F32)
                stdv = mp.tile([128, A], F32)
                mnew = mp.tile([128, A], F32)
                rU = rvqu[:, 60:61]
                nc.vector.reciprocal(inv[:], rU)
                for blk, (n0, nn) in enumerate(((0, 30), (30, 2))):
                    rsrc = rvqu if blk == 0 else rvqu2
                    rV = rsrc[:, 0 : 2 * nn : 2]
                    rQ = rsrc[:, 1 : 2 * nn : 2]
                    mhB = mh[:, n0 : n0 + nn]
                    qB = q[:, n0 : n0 + nn]
                    t2B = t2[:, n0 : n0 + nn]
                    msqB = msq[:, n0 : n0 + nn]
                    varB = var[:, n0 : n0 + nn]
                    stdB = mstd[:, 1, n0 : n0 + nn]
                    nc.vector.tensor_scalar(
                        t2B, rV, rU[:, 0:1], -1.0, op0=ALU.subtract, op1=ALU.mult
                    )
                    nc.vector.tensor_scalar(mhB, t2B, inv[:, 0:1], None, op0=ALU.mult)
                    nc.vector.tensor_scalar(t2B, rV, -2.0, None, op0=ALU.mult)
                    nc.vector.tensor_tensor(t2B, t2B, rQ, op=ALU.add)
                    nc.vector.tensor_scalar(t2B, t2B, rU[:, 0:1], None, op0=ALU.add)
                    nc.vector.tensor_scalar(qB, t2B, inv[:, 0:1], None, op0=ALU.mult)
                    nc.vector.tensor_tensor(msqB, mhB, mhB, op=ALU.mult)
                    nc.vector.tensor_tensor(varB, qB, msqB, op=ALU.subtract)
                    nc.vector.tensor_scalar(varB, varB, 0.0, None, op0=ALU.max)
                    nc.scalar.sqrt(stdB, varB)
                    nc.vector.tensor_scalar(
                        stdB, stdB, MIN_STD, 1.0, op0=ALU.max, op1=ALU.min
                    )
                    nc.vector.tensor_scalar(mhB, mhB, 1.0 - MOM, None, op0=ALU.mult)
                    nc.vector.scalar_tensor_tensor(
                        mstd[:, 0, n0 : n0 + nn], means_t[:, n0 : n0 + nn], MOM,
                        mhB, op0=ALU.mult, op1=ALU.add,
                    )
                nc.sync.dma_start(out_d[0, :, 0, :], mstd[:, 0, :])
                nc.sync.dma_start(out_d[1, :, 0, :], mstd[:, 1, :])
            else:
                dbg = mp.tile([128, A], F32)
                nc.vector.memset(dbg[:], 0.0)
                if stage >= 3:
                    nc.vector.tensor_copy(dbg[:, 0:4], w4[:])
                    nc.vector.tensor_copy(dbg[:, 4:8], rank4[:])
                else:
                    nc.vector.tensor_copy(dbg[:, 0:4], down32[:])
                nc.scalar.dma_start(out_d[0, :, 0, :], dbg[:])
                nc.scalar.dma_start(out_d[1, :, 0, :], dbg[:])

    nc.compile()
    return nc


def _get_nc(stage=None, single=None):
    if stage is None:
        stage = int(os.environ.get("CEM_STAGE", "9"))
    if single is None:
        single = bool(int(os.environ.get("CEM_SINGLE", "0")))
    key = ("nc", stage, single)
    if key not in _CACHE:
        _CACHE[key] = _build(stage, single)
    return _CACHE[key]


def kernel(**inputs):
    obs = np.ascontiguousarray(np.asarray(inputs["obs_diffs"], np.float32))
    means = np.ascontiguousarray(np.asarray(inputs["means"], np.float32))
    stds = np.ascontiguousarray(np.asarray(inputs["stds"], np.float32))
    noise = np.ascontiguousarray(np.asarray(inputs["noise"], np.float32))

    nc = _get_nc(stage=9, single=False)
    in_maps = []
    for c in range(NCORES):
        in_maps.append(
            {
                "obs": obs[c * PL : (c + 1) * PL],
                "means": means,
                "stds": stds,
                "noise": np.ascontiguousarray(noise[:, c * PL : (c + 1) * PL, :]),
            }
        )
    res = bass_utils.run_bass_kernel_spmd(
        nc, in_maps, core_ids=list(range(NCORES))
    )
    out = np.asarray(res.results[0]["out"], np.float32)
    return out.reshape(2, T, 1, A)
